# revision 1
# baseline (speedup 1.0000x reference)
"""BiMamba block kernel for 8 Trainium2 NeuronCores.

Sharding: core = 2*sample + direction (4 samples x 2 scan directions).
Each core runs the full mamba for its (sample, direction).

Structure: the input/gate projection and the causal depthwise conv are
fused into four accumulating matmuls (host-precomputed W_k = diag(
conv_w[:,k]) @ in_w), emitted per 512-column chunk so the chain to the
first scan is short.  The selective scan runs in four 1024-column
segments; per segment each of the 16 states does exp(dt*A) on ACT, the
dt*x*B product and DVE tensor_tensor_scan and C*h product on DVE in
bf16, with the state sum accumulated in PSUM by identity matmuls.  The
3x3 conv + pair AllReduce of the block tail runs in three waves tied to
segment completion so only the last wave's collective is exposed.
BatchNorm batch stats come from ACT accumulators (mean from conv
partials, var from pair-summed conv) with an 8-core AllReduce, then
residual + LeakyReLU.
"""
import os
import sys

for _p in ("/opt/trn_rl_repo", "/root/.axon_site/_ro/trn_rl_repo"):
    if os.path.isdir(_p):
        if _p not in sys.path:
            sys.path.insert(0, _p)
        break

import ml_dtypes
import numpy as np

# The agent image's antenv lacks axon_hooks; inject it so trace=True can
# capture NTFF profiles (used by test.py for HW timing, not for grading).
try:
    import antenv.axon_hooks  # noqa: F401
except ImportError:
    try:
        import types as _types

        from trn_agent_boot.trn_boot import _ntff_profile_via_ctypes

        _hook = _ntff_profile_via_ctypes("/opt/axon/libaxon_pjrt.so")
        _m = _types.ModuleType("antenv.axon_hooks")
        _m.get_axon_ntff_profile_hook = lambda: _hook
        _m.set_axon_ntff_profile_hook = lambda h: None
        sys.modules["antenv.axon_hooks"] = _m
    except Exception:
        pass

import concourse.bass as bass
import concourse.mybir as mybir
from concourse import bacc
from concourse import bass_utils
from concourse.masks import make_identity
from concourse.tile import TileContext

F32 = mybir.dt.float32
BF16 = mybir.dt.bfloat16
AF = mybir.ActivationFunctionType
OP = mybir.AluOpType

B, C, H, W = 4, 64, 64, 64
L = H * W          # 4096
DI = 128           # d_inner
DS = 16            # d_state
DTR = 4            # dt_rank
DCONV = 4
NCORE = 8
CH = 512           # matmul free-dim chunk
NCH = L // CH      # 8
SEGS = ((0, 2048), (2048, 4096))
NSEG = len(SEGS)
RPC = CH // W      # output rows per chunk (8)

BH_COLS = 9 * C + C + 128 + 32   # c3w | owT | bigT | bcwT
BF_COLS = 728


def _build():
    nc = bacc.Bacc(target_bir_lowering=False, debug=False, num_devices=NCORE)

    def din(name, shape, dtype=F32):
        return nc.dram_tensor(name, shape, dtype, kind="ExternalInput")

    F32R = mybir.dt.float32r
    x_loc = din("x_loc", [C, L], F32R)
    blob_f = din("blob_f", [128, BF_COLS], F32R)
    blob_h = din("blob_h", [128, BH_COLS], BF16)

    out_d = nc.dram_tensor("out", [C, L], F32, kind="ExternalOutput")

    with TileContext(nc) as tc:
        with tc.tile_pool(name="pers", bufs=1) as pers:
            # ---- params arrive as two packed blobs ----
            p_bf = pers.tile([128, BF_COLS], F32R)
            p_bh = pers.tile([128, BH_COLS], BF16)
            nc.sync.dma_start(p_bf[:], blob_f[:])
            nc.sync.dma_start(p_bh[:], blob_h[:])
            p_wk = [p_bf[:, 128 * k:128 * (k + 1)] for k in range(DCONV)]
            p_zwT = p_bf[:, 512:640]
            p_rwT = p_bf[:, 640:704]
            p_c1b = p_bf[:, 704:705].bitcast(F32)
            p_c1bh = p_bf[:, 727:728].bitcast(F32)
            p_dtb = p_bf[:, 705:706].bitcast(F32)
            p_A = p_bf[:, 706:722].bitcast(F32)
            p_D = p_bf[:, 722:723].bitcast(F32)
            p_c3b = p_bf[:C, 723:724].bitcast(F32)
            p_rb = p_bf[:C, 724:725].bitcast(F32)
            p_bng = p_bf[:C, 725:726].bitcast(F32)
            p_bnb = p_bf[:C, 726:727].bitcast(F32)
            p_c3w = p_bh[:, 0:9 * C]
            p_owT = p_bh[:, 9 * C:9 * C + C]
            p_bigT = p_bh[:, 9 * C + C:9 * C + C + 128]
            p_bcwT = p_bh[:, 9 * C + C + 128:9 * C + C + 160]

            ident = pers.tile([128, 128], F32)
            make_identity(nc, ident[:])
            ident_g = pers.tile([128, 128], BF16)
            nc.vector.tensor_copy(ident_g[:], ident[:])

            # DRAM staging for B/C rows (DMA partition-broadcast needs a
            # DRAM source)
            bc_dram = nc.dram_tensor("bc_stage", [2 * DS, L], BF16)

            x_pad = pers.tile([64, 3 + L], F32R)
            nc.gpsimd.memset(x_pad[:, 0:3].bitcast(F32), 0.0)
            nc.sync.dma_start(x_pad[:, 3:3 + L], x_loc[:])

            with tc.tile_pool(name="smid", bufs=1) as smid, \
                 tc.tile_pool(name="ps", bufs=8, space="PSUM") as psp, \
                 tc.tile_pool(name="sl_a", bufs=4) as pla, \
                 tc.tile_pool(name="sl_b", bufs=4) as plb, \
                 tc.tile_pool(name="sl_x", bufs=3) as plx, \
                 tc.tile_pool(name="sl_h", bufs=3) as plh, \
                 tc.tile_pool(name="sl_c", bufs=4) as plc, \
                 tc.tile_pool(name="sl_g", bufs=3) as plg, \
                 tc.tile_pool(name="sl_f", bufs=2) as plf, \
                 tc.tile_pool(name="dram", bufs=1, space="DRAM") as dr:
                z_sil = smid.tile([DI, L], BF16)
                dtv = smid.tile([DI, L], BF16)
                dtxc = smid.tile([DI, L], BF16)
                xcd = smid.tile([DI, L], BF16)
                xc = smid.tile([DI, L], BF16)
                carry = smid.tile([DI, DS], F32)

                ympad = smid.tile([128, H + 2, W + 2], BF16)
                nc.gpsimd.memset(ympad[:], 0.0)
                res_sb = smid.tile([C, L], BF16)
                conv_part = smid.tile([C, L], BF16)
                conv_full = smid.tile([C, L], BF16)
                stats_m = smid.tile([C, NCH], F32)
                stats_v = smid.tile([C, 4], F32)
                PAIRS = [[0, 1], [2, 3], [4, 5], [6, 7]]

                def front_proj(c):
                    # silu(v) computed as v*(1+tanh(v/2)) (times 0.5 folded
                    # into downstream weights on the host) -- Tanh shares the
                    # Exp table set, so no ACT table reloads mid-kernel
                    sl = slice(c * CH, (c + 1) * CH)
                    # fused in-projection + causal depthwise conv
                    ps = psp.tile([128, CH], F32, tag="ps")
                    for k in range(DCONV):
                        nc.tensor.matmul(ps[:DI], p_wk[k][:C],
                                         x_pad[:, c * CH + k:c * CH + k + CH],
                                         start=(k == 0), stop=(k == DCONV - 1))
                    th = plf.tile([DI, CH], BF16, tag="th")
                    nc.scalar.activation(th[:], ps[:DI], AF.Tanh,
                                         scale=0.5, bias=p_c1bh[:, 0:1])
                    raw = plf.tile([DI, CH], BF16, tag="raw")
                    nc.scalar.activation(raw[:], ps[:DI], AF.Identity,
                                         bias=p_c1b[:, 0:1])
                    nc.vector.scalar_tensor_tensor(
                        xc[:, sl], th[:], 1.0, raw[:],
                        op0=OP.add, op1=OP.mult)
                    # gate projection
                    ps2 = psp.tile([128, CH], F32, tag="ps")
                    nc.tensor.matmul(ps2[:DI], p_zwT[:C],
                                     x_pad[:, 3 + c * CH:3 + (c + 1) * CH],
                                     start=True, stop=True)
                    th2 = plf.tile([DI, CH], BF16, tag="th")
                    nc.scalar.activation(th2[:], ps2[:DI], AF.Tanh, scale=0.5)
                    raw2 = plf.tile([DI, CH], BF16, tag="raw")
                    nc.scalar.copy(raw2[:], ps2[:DI])
                    nc.vector.scalar_tensor_tensor(
                        z_sil[:, sl], th2[:], 1.0, raw2[:],
                        op0=OP.add, op1=OP.mult)
                    # dt projection (softplus finished by the batched Ln) and
                    # B/C projection -> DRAM staging
                    ps3 = psp.tile([128, CH], F32, tag="ps")
                    nc.tensor.matmul(ps3[:DI], p_bigT[:], xc[:, sl],
                                     start=True, stop=True)
                    nc.scalar.activation(dtv[:, sl], ps3[:DI], AF.Exp,
                                         bias=p_dtb[:, 0:1])
                    ps4 = psp.tile([128, CH], F32, tag="ps")
                    nc.tensor.matmul(ps4[:2 * DS], p_bcwT[:], xc[:, sl],
                                     start=True, stop=True)
                    bch = plb.tile([2 * DS, CH], BF16, tag="bch")
                    nc.vector.tensor_copy(bch[:], ps4[:2 * DS])
                    nc.sync.dma_start(bc_dram[:, sl], bch[:])

                def front_half(cs):
                    for c in cs:
                        front_proj(c)
                    hsl = slice(cs[0] * CH, (cs[-1] + 1) * CH)
                    nc.scalar.activation(dtv[:, hsl], dtv[:, hsl], AF.Ln,
                                         bias=1.0)
                    for c in cs:
                        sl = slice(c * CH, (c + 1) * CH)
                        nc.vector.tensor_mul(dtxc[:, sl], dtv[:, sl],
                                             xc[:, sl])
                        nc.scalar.activation(xcd[:, sl], xc[:, sl],
                                             AF.Copy, scale=p_D[:, 0:1])

                cc_ins, cc_outs = [], []
                WAVES = ((0, 1, 2), (3, 4, 5, 6, 7))
                for wi, cvs in enumerate(WAVES):
                    cc_ins.append(dr.tile([C, len(cvs) * CH], BF16,
                                          name=f"cci{wi}"))
                    cc_outs.append(dr.tile([C, len(cvs) * CH], BF16,
                                           name=f"cco{wi}"))
                st_in = dr.tile([C, 2], F32)
                st_out = nc.dram_tensor("st_out", [C, 2], F32,
                                        addr_space="Shared")

                def conv3_chunk(c):
                    ps = psp.tile([128, CH], F32, tag="ps", name=f"cv{c}")
                    ps3 = ps[:C].rearrange("p (r w) -> p r w", w=W)
                    r0 = c * RPC
                    n = 0
                    for ky in range(3):
                        for kx in range(3):
                            nc.tensor.matmul(
                                ps3[:],
                                p_c3w[:, (ky * 3 + kx) * C:
                                      (ky * 3 + kx + 1) * C],
                                ympad[:, r0 + ky:r0 + ky + RPC, kx:kx + W],
                                start=(n == 0), stop=(n == 8))
                            n += 1
                    # mean stat accumulates over the partial conv (linear in
                    # the pair sum, so the 8-core AllReduce recovers it)
                    nc.scalar.activation(conv_part[:, c * CH:(c + 1) * CH],
                                         ps3.rearrange("p r w -> p (r w)"),
                                         AF.Identity, bias=p_c3b[:, 0:1],
                                         accum_out=stats_m[:, c:c + 1])

                def do_wave(wi):
                    cvs = WAVES[wi]
                    for c in cvs:
                        conv3_chunk(c)
                    lo, hi = cvs[0] * CH, (cvs[-1] + 1) * CH
                    nc.sync.dma_start(cc_ins[wi][:], conv_part[:, lo:hi])
                    nc.gpsimd.collective_compute(
                        "AllReduce", OP.add, replica_groups=PAIRS,
                        ins=[cc_ins[wi][:].opt()], outs=[cc_outs[wi][:].opt()])
                    nc.sync.dma_start(conv_full[:, lo:hi], cc_outs[wi][:])
                    # square scratch overwrites conv_part (dead after the
                    # collective input DMA); only accum_out is consumed
                    nc.scalar.activation(conv_part[:, lo:hi],
                                         conv_full[:, lo:hi],
                                         AF.Square,
                                         accum_out=stats_v[:, wi:wi + 1])

                front_half((0, 1, 2, 3))
                front_half((4, 5, 6, 7))

                for q, (t0, t1) in enumerate(SEGS):
                    SEG = t1 - t0
                    qsl = slice(t0, t1)
                    chunks = range(t0 // CH, t1 // CH)
                    y_ps = {}
                    for cix in chunks:
                        yp = psp.tile([128, CH], F32, tag="ps",
                                      name=f"y{cix}")
                        nc.tensor.matmul(yp[:DI], ident_g[:],
                                         xcd[:, cix * CH:(cix + 1) * CH],
                                         start=True, stop=False)
                        y_ps[cix] = yp
                    for s in range(DS):
                        da = pla.tile([DI, SEG], BF16, tag="da")
                        nc.scalar.activation(da[:], dtv[:, qsl], AF.Exp,
                                             scale=p_A[:, s:s + 1])
                        bbc = plb.tile([DI, SEG], BF16, tag="bbc")
                        nc.sync.dma_start(
                            bbc[:],
                            bc_dram[s:s + 1, qsl].to_broadcast((DI, SEG)))
                        dbx = plx.tile([DI, SEG], BF16, tag="dbx")
                        nc.vector.tensor_mul(dbx[:], dtxc[:, qsl], bbc[:])
                        h = plh.tile([DI, SEG], BF16, tag="h")
                        init = 0.0 if q == 0 else carry[:, s:s + 1]
                        nc.vector.tensor_tensor_scan(h[:], da[:], dbx[:],
                                                     init, op0=OP.mult,
                                                     op1=OP.add)
                        if q < NSEG - 1:
                            # on DVE so the in-order ACT queue of exps is
                            # never blocked behind a scan result
                            nc.vector.tensor_copy(carry[:, s:s + 1],
                                                  h[:, SEG - 1:SEG])
                        cbc = plc.tile([DI, SEG], BF16, tag="cbc")
                        nc.sync.dma_start(
                            cbc[:],
                            bc_dram[DS + s:DS + s + 1, qsl].to_broadcast(
                                (DI, SEG)))
                        g = plg.tile([DI, SEG], BF16, tag="g")
                        nc.vector.tensor_mul(g[:], h[:], cbc[:])
                        for j, cix in enumerate(chunks):
                            nc.tensor.matmul(y_ps[cix][:DI], ident_g[:],
                                             g[:, j * CH:(j + 1) * CH],
                                             start=False, stop=(s == DS - 1))

                    # gating + out-projection + padded spatial write +
                    # residual for this segment's chunks
                    for cix in chunks:
                        sl = slice(cix * CH, (cix + 1) * CH)
                        yg = plf.tile([DI, CH], BF16, tag="yg")
                        nc.vector.tensor_mul(yg[:], y_ps[cix][:DI],
                                             z_sil[:, sl])
                        po = psp.tile([128, CH], F32, tag="ps",
                                      name=f"po{cix}")
                        nc.tensor.matmul(po[:C], p_owT[:], yg[:],
                                         start=True, stop=True)
                        r0 = cix * RPC
                        nc.scalar.copy(
                            ympad[0:C, 1 + r0:1 + r0 + RPC, 1:1 + W],
                            po[:C].rearrange("p (r w) -> p r w", w=W))
                        psr = psp.tile([128, CH], F32, tag="ps",
                                       name=f"rs{cix}")
                        nc.tensor.matmul(psr[:C], p_rwT[:C],
                                         x_pad[:, 3 + cix * CH:
                                               3 + (cix + 1) * CH],
                                         start=True, stop=True)
                        nc.scalar.activation(res_sb[:, sl], psr[:C],
                                             AF.Identity, bias=p_rb[:, 0:1])

                    do_wave(q)

                # ---- batch stats AllReduce + BN + residual + leaky ----
                tl = smid
                stats = tl.tile([C, 2], F32)
                nc.vector.tensor_reduce(stats[:, 0:1], stats_m[:],
                                        axis=mybir.AxisListType.X, op=OP.add)
                nc.vector.tensor_add(stats[:, 1:2], stats_v[:, 0:1],
                                     stats_v[:, 1:2])
                nc.sync.dma_start(st_in[:], stats[:])
                nc.gpsimd.collective_compute(
                    "AllReduce", OP.add,
                    replica_groups=[[0, 1, 2, 3, 4, 5, 6, 7]],
                    ins=[st_in[:].opt()], outs=[st_out[:].opt()])
                stot = tl.tile([C, 2], F32)
                nc.sync.dma_start(stot[:], st_out[:])

                # mean counts each position once; var partials double count
                # (both pair cores sum the identical pair-summed conv)
                inv1 = 1.0 / (B * L)
                inv2 = 1.0 / (2.0 * B * L)
                mean = tl.tile([C, 1], F32)
                var = tl.tile([C, 1], F32)
                tmp = tl.tile([C, 1], F32)
                nc.vector.tensor_scalar_mul(mean[:], stot[:, 0:1], inv1)
                nc.vector.tensor_scalar_mul(var[:], stot[:, 1:2], inv2)
                nc.vector.tensor_mul(tmp[:], mean[:], mean[:])
                nc.vector.tensor_sub(var[:], var[:], tmp[:])
                # invstd = 1/sqrt(var + eps)
                nc.vector.tensor_scalar_add(var[:], var[:], 1e-5)
                nc.scalar.activation(tmp[:], var[:], AF.Sqrt)
                nc.vector.reciprocal(tmp[:], tmp[:])
                scal = tl.tile([C, 1], F32)
                shft = tl.tile([C, 1], F32)
                nc.vector.tensor_mul(scal[:], p_bng[:], tmp[:])
                nc.vector.tensor_mul(tmp[:], mean[:], scal[:])
                nc.vector.tensor_sub(shft[:], p_bnb[:], tmp[:])

                # bn + residual + leaky relu:
                #   out = prelu(conv*scal + res + shft)
                for wi, (lo, hi) in enumerate(((0, CH * 2), (CH * 2, CH * 4),
                                               (CH * 4, CH * 6), (CH * 6, L))):
                    bn1 = plf.tile([C, CH * 2], F32, tag="bn")
                    nc.vector.scalar_tensor_tensor(
                        bn1[:], conv_full[:, lo:hi],
                        scal[:, 0:1], res_sb[:, lo:hi],
                        op0=OP.mult, op1=OP.add)
                    nc.scalar.activation(bn1[:], bn1[:],
                                         AF.Prelu, alpha=0.01,
                                         bias=shft[:, 0:1])
                    nc.sync.dma_start(out_d[:, lo:hi], bn1[:])

    nc.compile()
    return nc


_NC = None


def _get_nc():
    global _NC
    if _NC is None:
        _NC = _build()
    return _NC


def _prep_in_maps(inp):
    inp = {k: np.asarray(v, dtype=np.float32) for k, v in inp.items()}
    x = inp["x"]  # (4, 64, 64, 64)
    maps = []
    for core in range(NCORE):
        b, d = core // 2, core % 2
        pre = "m1_" if d == 0 else "m2_"
        in_w = inp[pre + "in_w"]          # (256, 64)
        xproj_w = inp[pre + "xproj_w"]    # (36, 128)
        dt_w = inp[pre + "dt_w"]          # (128, 4)
        conv1_w = inp[pre + "conv_w"]     # (128, 4)

        x_loc = x[b].reshape(C, L)
        if d == 1:
            x_loc = x_loc[:, ::-1]

        # the tanh-form silu leaves xc and z scaled by 2; fold the halves
        # into the consuming weights
        bigproj = 0.5 * (dt_w @ xproj_w[:DTR])    # (128, 128)
        conv3_slice = inp["conv_w"][:, d * C:(d + 1) * C]  # (64,64,3,3)
        c3 = np.zeros((128, 9 * C), np.float32)
        for ky in range(3):
            for kx in range(3):
                c3[:C, (ky * 3 + kx) * C:(ky * 3 + kx + 1) * C] = \
                    conv3_slice[:, :, ky, kx].T

        blob_f = np.zeros((128, BF_COLS), np.float32)
        # fused in-projection + depthwise causal conv:
        # W_k[ch_x, di] = in_w[di, ch_x] * conv1_w[di, k]
        xi_w = in_w[:DI]                  # (128, 64)
        for k in range(DCONV):
            blob_f[:C, 128 * k:128 * (k + 1)] = \
                (xi_w * conv1_w[:, k:k + 1]).T
        blob_f[:C, 512:640] = in_w[DI:].T
        blob_f[:C, 640:704] = inp["res_w"].T
        blob_f[:, 704] = inp[pre + "conv_b"]
        blob_f[:, 705] = inp[pre + "dt_b"]
        blob_f[:, 706:722] = -np.exp(inp[pre + "A_log"])
        blob_f[:, 722] = inp[pre + "D"]
        blob_f[:C, 723] = (inp["conv_b"] if d == 0
                           else np.zeros_like(inp["conv_b"]))
        blob_f[:C, 724] = inp["res_b"]
        blob_f[:C, 725] = inp["bn_gamma"]
        blob_f[:C, 726] = inp["bn_beta"]
        blob_f[:, 727] = 0.5 * inp[pre + "conv_b"]
        blob_h = np.zeros((128, BH_COLS), np.float32)
        blob_h[:, 0:9 * C] = c3
        blob_h[:, 9 * C:9 * C + C] = 0.25 * inp[pre + "out_w"].T
        blob_h[:, 9 * C + C:9 * C + C + 128] = bigproj.T
        blob_h[:, 9 * C + C + 128:9 * C + C + 160] = 0.5 * xproj_w[DTR:].T
        m = {
            "x_loc": np.ascontiguousarray(x_loc),
            "blob_f": blob_f,
            "blob_h": blob_h.astype(ml_dtypes.bfloat16),
        }
        maps.append(m)
    return maps


def _run(inputs, trace=False):
    nc = _get_nc()
    maps = _prep_in_maps(inputs)
    res = bass_utils.run_bass_kernel_spmd(
        nc, maps, core_ids=list(range(NCORE)), trace=trace)
    out = np.stack([res.results[2 * b]["out"].reshape(C, H, W)
                    for b in range(B)])
    return out, res


def kernel(**inputs) -> np.ndarray:
    out, _ = _run(inputs, trace=False)
    return out

